# revision 1
# baseline (speedup 1.0000x reference)
"""GCN (2-layer + mean-pool + classifier) Bass/Tile kernel for 8 Trainium2
NeuronCores, self-contained.

Sharding: dst-node partitioning (12544 padded nodes / 98 windows of 128 per
core). Per layer, aggregation y[d] = sum_{e: dst=d} u[src_e] (+ self) runs as
PE one-hot matmuls over edge tiles:
  gather  G = S_src(fp8).T @ U_block(bf16)   (per (wgroup, block), PSUM)
  staged to HBM, per-window strided-DMA "bucket transpose", then
  scatter  acc(CH,128) += G2_j.T @ S_dst_j(fp8)  (49 tiles / window)
plus spill tiles (indirect-DMA row gathers) and a self-loop matmul.
Between layers the node table is AllGathered; pooling uses per-window one-hot
matmuls into a persistent PSUM accumulator + AllReduce; the classifier runs
on-chip. Output [512, 2] f32 (identical on every core).

Edge bucketing: (784 src blocks x 96 windows) x 8 slots; bucket overflow and
windows 96/97 go to the spill path (fixed 260 spill tiles / core).
"""
import numpy as np
import ml_dtypes

# ---------------------------------------------------------------- constants
N = 100000
N_PAD = 100352
NBLK = 784
NCORES = 8
WPC = 98
NWG = 6
B = 8
NGT = NBLK * NWG
NST = WPC * 49
SPILL_CAP = [2] * 96 + [34, 34]
NSPILL = sum(SPILL_CAP)
SPILL_T0 = np.concatenate([[0], np.cumsum(SPILL_CAP)]).astype(int)
NGRAPH = 512
FP8NP = ml_dtypes.float8_e4m3
BF16NP = ml_dtypes.bfloat16

_LAST_RESULTS = None


# ------------------------------------------------------------------ patches
def _install_patches():
    import json

    import concourse.mybir as mybir
    import concourse.tile as tile_mod
    from concourse.vector_clock import ScopedClock

    if not getattr(tile_mod.TileContext, "_gcn_patched", False):
        def _drain_and_barrier(self, tick_clock, wait_clock):
            nc = self.nc
            drain_inst = nc.sync.drain()
            wait_clock.add_sem_waits(
                drain_inst.ins, ScopedClock({None: tick_clock.global_clock}))
            si = drain_inst.ins.sync_info
            waits = list(si.on_wait) if si is not None and si.on_wait else []
            if len(waits) > 1:
                si.on_wait = waits[:1]
                for w in waits[1:]:
                    extra = nc.sync.drain()
                    extra.ins.sync_info = mybir.SyncInfo(
                        on_wait=[w], on_update=[])
            nc.all_engine_barrier()
            assert self.sems is not None
            popped = nc._tile_sem_poison_stack.pop()
            assert popped is self._sem_poison
            nc.clear_and_free_semaphores(list(self.sems.allocated().values()))
            nc.all_engine_barrier()

        tile_mod.TileContext._drain_and_barrier = _drain_and_barrier
        tile_mod.TileContext._gcn_patched = True

    import concourse.bass as bass_mod

    if not getattr(bass_mod.Bass, "_wait_split_patched", False):
        orig = bass_mod.Bass.to_json_bytes

        def _split(data):
            j = json.loads(data)
            cnt = [0]

            def fix(insts):
                out = []
                for inst in insts:
                    si = inst.get("sync_info")
                    waits = si.get("on_wait") if si else None
                    if waits and len(waits) > 1:
                        for w in waits[:-1]:
                            cnt[0] += 1
                            out.append({
                                "debug": inst.get("debug", 0),
                                "engine": inst["engine"],
                                "ins": [], "outs": [],
                                "name": f"WSPL-{cnt[0]}-{inst['name']}",
                                "opcode": "EventSemaphore",
                                "sync_info": {"on_update": [], "on_wait": [w]},
                            })
                        si["on_wait"] = [waits[-1]]
                    out.append(inst)
                insts[:] = out

            def walk(d):
                if isinstance(d, dict):
                    for k, v in d.items():
                        if k == "instructions" and isinstance(v, list):
                            fix(v)
                        else:
                            walk(v)
                elif isinstance(d, list):
                    for e in d:
                        walk(e)

            walk(j)
            return json.dumps(j).encode()

        def to_json_bytes(self, *a, **kw):
            return _split(orig(self, *a, **kw))

        bass_mod.Bass.to_json_bytes = to_json_bytes
        bass_mod.Bass._wait_split_patched = True


# ----------------------------------------------------------------- cpu prep
def _prepare(x, edge_index, batch, W1, b1, W2, b2, Wc, bc):
    src = np.asarray(edge_index[0], dtype=np.int64)
    dst = np.asarray(edge_index[1], dtype=np.int64)
    batch = np.asarray(batch, dtype=np.int64)
    x = np.asarray(x, dtype=np.float32)

    deg = np.ones(N_PAD, dtype=np.float32)
    np.add.at(deg, dst, 1.0)
    dinv = (1.0 / np.sqrt(deg)).astype(np.float32)

    x_pad = np.zeros((N_PAD, 8), dtype=np.float32)
    x_pad[:N] = x
    x_blk3 = x_pad.reshape(NBLK, 128, 8).transpose(1, 0, 2)  # [rel, blk, ch]
    x_blk = np.ascontiguousarray(x_blk3).astype(BF16NP)
    dinv_pb = np.ascontiguousarray(dinv.reshape(NBLK, 128).T)  # [rel, blk]

    eye_fp8 = np.eye(128, dtype=np.float32).astype(FP8NP)
    eye_bf16 = np.eye(128, dtype=np.float32).astype(BF16NP)
    eye_f32 = np.eye(128, dtype=np.float32)

    cnt = np.zeros(NGRAPH, dtype=np.float32)
    np.add.at(cnt, batch, 1.0)
    cnt_inv = np.where(cnt > 0, 1.0 / np.maximum(cnt, 1.0), 1.0).astype(np.float32)
    cnt_inv = np.ascontiguousarray(cnt_inv.reshape(4, 128).T)[:, :, None].copy()

    order = np.argsort(dst, kind="stable")
    src_s, dst_s = src[order], dst[order]
    core_of = dst_s // (WPC * 128)
    bounds = np.searchsorted(core_of, np.arange(NCORES + 1))

    per_core = []
    for c in range(NCORES):
        lo, hi = bounds[c], bounds[c + 1]
        es, ed = src_s[lo:hi], dst_s[lo:hi]
        w = (ed // 128) - c * WPC
        I = es // 128
        rs = es % 128
        rd = ed % 128

        o2 = np.lexsort((I, w))
        w2, I2, rs2, rd2 = w[o2], I[o2], rs[o2], rd[o2]
        key = w2 * NBLK + I2
        _, start, cnts = np.unique(key, return_index=True, return_counts=True)
        rank = np.arange(len(key)) - np.repeat(start, cnts)

        main = (w2 < 96) & (rank < B)
        spm = ~main

        wm, Im, rsm, rdm, bm = (a[main] for a in (w2, I2, rs2, rd2, rank))
        g = wm // 16
        w_lo = wm % 16
        I_lo, I_hi = Im % 16, Im // 16
        gt = g * NBLK + (I_lo * 49 + I_hi)
        slot = w_lo * 8 + bm
        s_src = np.zeros((128, NGT, 128), dtype=FP8NP)
        s_src[rsm, gt, slot] = 1.0
        st = wm * 49 + I_hi
        sp_part = bm * 16 + I_lo
        s_dst = np.zeros((128, NST, 128), dtype=FP8NP)
        s_dst[sp_part, st, rdm] = 1.0

        ws, Is, rss, rds = (a[spm] for a in (w2, I2, rs2, rd2))
        o3 = np.argsort(ws, kind="stable")
        ws, Is, rss, rds = ws[o3], Is[o3], rss[o3], rds[o3]
        wstart = np.searchsorted(ws, np.arange(WPC + 1))
        spill_idx = np.zeros((128, NSPILL), dtype=np.int32)
        s_spill = np.zeros((128, NSPILL, 128), dtype=FP8NP)
        for wi in range(WPC):
            a, bnd = wstart[wi], wstart[wi + 1]
            nsp = bnd - a
            if nsp > SPILL_CAP[wi] * 128:
                raise RuntimeError(
                    f"core {c} window {wi}: spill {nsp} > {SPILL_CAP[wi]*128}")
            k = np.arange(nsp)
            t = SPILL_T0[wi] + k // 128
            p = k % 128
            spill_idx[p, t] = (rss[a:bnd] * NBLK + Is[a:bnd]).astype(np.int32)
            s_spill[p, t, rds[a:bnd]] = 1.0

        s_pool = np.zeros((128, WPC * 4, 128), dtype=FP8NP)
        base = c * WPC * 128
        nodes = np.arange(base, base + WPC * 128)
        valid = nodes < N
        gids = batch[np.minimum(nodes, N - 1)]
        wv = (nodes - base) // 128
        pv = (nodes - base) % 128
        s_pool[pv[valid], wv[valid] * 4 + (gids[valid] // 128),
               gids[valid] % 128] = 1.0

        dinv_rep = np.broadcast_to(
            dinv[base: base + WPC * 128].reshape(1, WPC, 128),
            (32, WPC, 128)).copy()
        blk0 = c * WPC
        x_self = np.ascontiguousarray(x_blk3[:, blk0:blk0 + WPC, :]).astype(BF16NP)
        dinv_self = np.ascontiguousarray(
            dinv_pb[:, blk0:blk0 + WPC])[:, :, None].copy()

        per_core.append({
            "x_blk": x_blk,
            "dinv_blk": np.ascontiguousarray(dinv_pb)[:, :, None].copy(),
            "dinv_rep": dinv_rep,
            "x_self": x_self, "dinv_self": dinv_self,
            "w1": np.asarray(W1, np.float32).astype(BF16NP),
            "b1": np.asarray(b1, np.float32).reshape(32, 1).copy(),
            "w2": np.asarray(W2, np.float32).astype(BF16NP),
            "b2": np.asarray(b2, np.float32).reshape(32, 1).copy(),
            "wc": np.asarray(Wc, np.float32).copy(),
            "bc": np.asarray(bc, np.float32).reshape(2, 1).copy(),
            "s_src": s_src, "s_dst": s_dst, "s_spilldst": s_spill,
            "spill_idx": spill_idx, "s_pool": s_pool,
            "eye_fp8": eye_fp8, "eye_bf16": eye_bf16, "eye_f32": eye_f32,
            "cnt_inv": cnt_inv,
        })
    return per_core


# ------------------------------------------------------------------ builder
def _build_nc():
    import concourse.bass as bass
    import concourse.mybir as mybir
    from concourse.tile import TileContext

    FP8 = mybir.dt.float8e4
    BF16 = mybir.dt.bfloat16
    F32 = mybir.dt.float32
    I32 = mybir.dt.int32
    AF = mybir.ActivationFunctionType

    nc = bass.Bass(target_bir_lowering=True)

    def inp(name, shape, dt):
        return nc.dram_tensor(name, shape, dt, kind="ExternalInput")

    x_blk = inp("x_blk", [128, NBLK, 8], BF16)
    dinv_blk = inp("dinv_blk", [128, NBLK, 1], F32)
    dinv_rep = inp("dinv_rep", [32, WPC, 128], F32)
    x_self = inp("x_self", [128, WPC, 8], BF16)
    dinv_self = inp("dinv_self", [128, WPC, 1], F32)
    w1 = inp("w1", [8, 32], BF16)
    b1 = inp("b1", [32, 1], F32)
    w2 = inp("w2", [32, 32], BF16)
    b2 = inp("b2", [32, 1], F32)
    wc = inp("wc", [32, 2], F32)
    bc = inp("bc", [2, 1], F32)
    s_src = inp("s_src", [128, NGT, 128], FP8)
    s_dst = inp("s_dst", [128, NST, 128], FP8)
    s_spill = inp("s_spilldst", [128, NSPILL, 128], FP8)
    spill_idx = inp("spill_idx", [128, NSPILL], I32)
    s_pool = inp("s_pool", [128, WPC * 4, 128], FP8)
    eye_fp8 = inp("eye_fp8", [128, 128], FP8)
    eye_bf16 = inp("eye_bf16", [128, 128], BF16)
    eye_f32 = inp("eye_f32", [128, 128], F32)
    cnt_inv = inp("cnt_inv", [128, 4, 1], F32)
    out = nc.dram_tensor("out", [NGRAPH, 2], F32, kind="ExternalOutput")

    u1_dram = nc.dram_tensor("u1_dram", [128 * NBLK, 8], BF16)
    u2_dram = nc.dram_tensor("u2_dram", [128 * NBLK, 32], BF16)
    g1_dram8 = nc.dram_tensor("g1_dram8", [128, NBLK, 8], BF16)
    g1_dram32 = nc.dram_tensor("g1_dram32", [128, NBLK, 32], BF16)
    u2_loc = nc.dram_tensor("u2_loc", [128, WPC * 32], BF16)
    u2_gath = nc.dram_tensor("u2_gath", [NCORES, 128, WPC * 32], BF16)
    pool_in = nc.dram_tensor("pool_in", [NGRAPH, 32], F32)
    pool_out = nc.dram_tensor("pool_out", [NGRAPH, 32], F32)

    groups = [list(range(NCORES))]

    with TileContext(nc) as tc:
        with tc.tile_pool(name="glob", bufs=1) as gl, \
             tc.tile_pool(name="gpsum", bufs=2, space="PSUM") as pp, \
             tc.tile_pool(name="accp", bufs=2, space="PSUM") as ap, \
             tc.tile_pool(name="hps", bufs=2, space="PSUM") as hp, \
             tc.tile_pool(name="trps", bufs=1, space="PSUM") as tp2, \
             tc.tile_pool(name="poolps", bufs=1, space="PSUM") as plp:

            def load(t, src_ap):
                nc.sync.dma_start(out=t[:], in_=src_ap[:])
                return t

            dinv_blk_t = load(gl.tile([128, NBLK, 1], F32, tag="dinvblk", name="dinvblk"), dinv_blk)
            dinv_rep_t = load(gl.tile([32, WPC, 128], F32, tag="dinvrep", name="dinvrep"), dinv_rep)
            dinv_self_t = load(gl.tile([128, WPC, 1], F32, tag="dinvself", name="dinvself"), dinv_self)
            eye8_t = load(gl.tile([128, 128], FP8, tag="eye8", name="eye8"), eye_fp8)
            eyeb_t = load(gl.tile([128, 128], BF16, tag="eyeb", name="eyeb"), eye_bf16)
            eyef_t = load(gl.tile([128, 128], F32, tag="eyef", name="eyef"), eye_f32)
            w1_t = load(gl.tile([8, 32], BF16, tag="w1t", name="w1t"), w1)
            b1_t = load(gl.tile([32, 1], F32, tag="b1t", name="b1t"), b1)
            w2_t = load(gl.tile([32, 32], BF16, tag="w2t", name="w2t"), w2)
            b2_t = load(gl.tile([32, 1], F32, tag="b2t", name="b2t"), b2)
            wc_t = load(gl.tile([32, 2], F32, tag="wct", name="wct"), wc)
            bc_t = load(gl.tile([2, 1], F32, tag="bct", name="bct"), bc)
            cnt_inv_t = load(gl.tile([128, 4, 1], F32, tag="cntt", name="cntt"), cnt_inv)
            spill_idx_t = load(gl.tile([128, NSPILL], I32, tag="spidx", name="spidx"), spill_idx)

            u2_self_t = gl.tile([128, WPC, 32], BF16)
            u2_t = gl.tile([128, NBLK, 32], BF16)

            def agg_layer(tag, u_t, u_self_ap, u_dram, g1_dram, CH, PB,
                          finalize, dbg=False):
                with tc.tile_pool(name=f"L{tag}", bufs=2) as lp, \
                     tc.tile_pool(name=f"L{tag}g2", bufs=3) as g2p, \
                     tc.tile_pool(name=f"L{tag}sp", bufs=1) as spp:
                    spillG = spp.tile([128, NSPILL, CH], BF16)
                    for t in range(NSPILL):
                        nc.gpsimd.indirect_dma_start(
                            out=spillG[:, t, :], out_offset=None,
                            in_=u_dram[:],
                            in_offset=bass.IndirectOffsetOnAxis(
                                ap=spill_idx_t[:, t:t + 1], axis=0))

                    def spill_self(w, acc, first):
                        t0, t1 = int(SPILL_T0[w]), int(SPILL_T0[w + 1])
                        if t1 > t0:
                            ssp = lp.tile([128, 34, 128], FP8, tag="sspill")
                            nc.sync.dma_start(out=ssp[:, :t1 - t0, :],
                                              in_=s_spill[:, t0:t1, :])
                            for k in range(t1 - t0):
                                nc.tensor.matmul(
                                    out=acc[:], lhsT=spillG[:, t0 + k, :],
                                    rhs=ssp[:, k, :],
                                    start=(first and k == 0), stop=False,
                                    skip_group_check=True)
                        nc.tensor.matmul(
                            out=acc[:], lhsT=u_self_ap[:, w, :], rhs=eye8_t[:],
                            start=False, stop=True, skip_group_check=True)

                    for g in range(NWG):
                        for ch0 in range(0, NBLK, PB):
                            n = min(PB, NBLK - ch0)
                            ssrc = lp.tile([128, PB, 128], FP8, tag="ssrc")
                            nc.sync.dma_start(
                                out=ssrc[:, :n, :],
                                in_=s_src[:, g * NBLK + ch0:
                                          g * NBLK + ch0 + n, :])
                            gbank = pp.tile([128, PB, CH], F32, tag="gbank")
                            for i in range(n):
                                Ip = ch0 + i
                                I = (Ip % 49) * 16 + Ip // 49
                                nc.tensor.matmul(
                                    out=gbank[:, i, :], lhsT=ssrc[:, i, :],
                                    rhs=u_t[:, I, :], start=True, stop=True,
                                    skip_group_check=True)
                            stag = lp.tile([128, PB, CH], BF16, tag="stag")
                            nc.vector.tensor_copy(out=stag[:, :n, :],
                                                  in_=gbank[:, :n, :])
                            nc.sync.dma_start(out=g1_dram[:, ch0:ch0 + n, :],
                                              in_=stag[:, :n, :])
                        for w_lo in range(16):
                            w = g * 16 + w_lo
                            G2 = g2p.tile([128, 49, CH], BF16, tag="g2")
                            for bb in range(8):
                                nc.sync.dma_start(
                                    out=G2[16 * bb:16 * (bb + 1), :, :],
                                    in_=g1_dram[8 * w_lo + bb, :, :].rearrange(
                                        "(il ih) c -> il ih c", il=16))
                            sdst = lp.tile([128, 49, 128], FP8, tag="sdst")
                            nc.sync.dma_start(
                                out=sdst[:],
                                in_=s_dst[:, w * 49:(w + 1) * 49, :])
                            acc = ap.tile([CH, 128], F32, tag="acc")
                            for j in range(49):
                                nc.tensor.matmul(
                                    out=acc[:], lhsT=G2[:, j, :],
                                    rhs=sdst[:, j, :], start=(j == 0),
                                    stop=False, skip_group_check=True)
                            spill_self(w, acc, first=False)
                            finalize(w, acc, lp)
                    for w in (96, 97):
                        acc = ap.tile([CH, 128], F32, tag="acc")
                        spill_self(w, acc, first=True)
                        finalize(w, acc, lp)

            # ---------------- layer 1
            with tc.tile_pool(name="u1p", bufs=1) as u1p:
                x_t = u1p.tile([128, NBLK, 8], BF16)
                nc.sync.dma_start(out=x_t[:], in_=x_blk[:])
                u1_t = u1p.tile([128, NBLK, 8], BF16)
                nc.vector.tensor_tensor(
                    out=u1_t[:], in0=x_t[:],
                    in1=dinv_blk_t[:].to_broadcast([128, NBLK, 8]),
                    op=mybir.AluOpType.mult)
                nc.sync.dma_start(
                    out=u1_dram[:].rearrange("(p b) c -> p b c", p=128),
                    in_=u1_t[:])
                xs_t = u1p.tile([128, WPC, 8], BF16)
                nc.sync.dma_start(out=xs_t[:], in_=x_self[:])
                u1_self_t = u1p.tile([128, WPC, 8], BF16)
                nc.vector.tensor_tensor(
                    out=u1_self_t[:], in0=xs_t[:],
                    in1=dinv_self_t[:].to_broadcast([128, WPC, 8]),
                    op=mybir.AluOpType.mult)

                def fin1(w, acc, lp):
                    z1 = lp.tile([8, 128], BF16, tag="z1")
                    nc.vector.tensor_tensor(
                        out=z1[:], in0=acc[:], in1=dinv_rep_t[0:8, w, :],
                        op=mybir.AluOpType.mult)
                    h1ps = hp.tile([32, 128], F32, tag="hps")
                    nc.tensor.matmul(out=h1ps[:], lhsT=w1_t[:], rhs=z1[:],
                                     start=True, stop=True,
                                     skip_group_check=True)
                    h1T = lp.tile([32, 128], BF16, tag="h1T")
                    nc.scalar.activation(out=h1T[:], in_=h1ps[:],
                                         func=AF.Relu, bias=b1_t[:], scale=1.0)
                    trps = tp2.tile([128, 32], BF16, tag="trp")
                    nc.tensor.transpose(out=trps[:], in_=h1T[:],
                                        identity=eyeb_t[:32, :32])
                    nc.vector.tensor_scalar(
                        out=u2_self_t[:, w, :], in0=trps[:],
                        scalar1=dinv_self_t[:, w, :], scalar2=None,
                        op0=mybir.AluOpType.mult)

                agg_layer("1", u1_t, u1_self_t, u1_dram, g1_dram8, 8, 64,
                          fin1, dbg=True)

            # ---------------- allgather u2
            nc.sync.dma_start(
                out=u2_loc[:].rearrange("p (w c) -> p w c", c=32),
                in_=u2_self_t[:])
            nc.gpsimd.collective_compute(
                "AllGather", mybir.AluOpType.bypass, replica_groups=groups,
                ins=[u2_loc.ap().opt()], outs=[u2_gath.ap().opt()])
            for c in range(NCORES):
                nc.sync.dma_start(
                    out=u2_t[:, c * WPC:(c + 1) * WPC, :],
                    in_=u2_gath[c].rearrange("p (w c) -> p w c", c=32))
            nc.sync.dma_start(
                out=u2_dram[:].rearrange("(p b) c -> p b c", p=128),
                in_=u2_t[:])

            # ---------------- layer 2 + pooling
            pool_acc = gl.tile([128, 4, 32], F32, tag="poolacc",
                               name="poolacc")
            nc.vector.memset(pool_acc[:], 0.0)

            def fin2(w, acc, lp):
                z2 = lp.tile([32, 128], BF16, tag="z2")
                nc.vector.tensor_tensor(
                    out=z2[:], in0=acc[:], in1=dinv_rep_t[:, w, :],
                    op=mybir.AluOpType.mult)
                h2ps = hp.tile([32, 128], F32, tag="hps")
                nc.tensor.matmul(out=h2ps[:], lhsT=w2_t[:], rhs=z2[:],
                                 start=True, stop=True, skip_group_check=True)
                h2T = lp.tile([32, 128], BF16, tag="h2T")
                nc.scalar.activation(out=h2T[:], in_=h2ps[:], func=AF.Relu,
                                     bias=b2_t[:], scale=1.0)
                trps = tp2.tile([128, 32], BF16, tag="trp")
                nc.tensor.transpose(out=trps[:], in_=h2T[:],
                                    identity=eyeb_t[:32, :32])
                h2sb = lp.tile([128, 32], BF16, tag="h2sb")
                nc.vector.tensor_copy(out=h2sb[:], in_=trps[:])
                spt = lp.tile([128, 4, 128], FP8, tag="spool")
                nc.sync.dma_start(out=spt[:],
                                  in_=s_pool[:, w * 4:(w + 1) * 4, :])
                pool_ps = plp.tile([128, 4, 32], F32, tag="poolps",
                                   name="poolps")
                for grp in range(4):
                    nc.tensor.matmul(
                        out=pool_ps[:, grp, :], lhsT=spt[:, grp, :],
                        rhs=h2sb[:], start=True, stop=True,
                        skip_group_check=True)
                nc.vector.tensor_tensor(
                    out=pool_acc[:], in0=pool_acc[:], in1=pool_ps[:],
                    op=mybir.AluOpType.add)

            agg_layer("2", u2_t, u2_self_t, u2_dram, g1_dram32, 32, 16, fin2)

            # ---------------- head
            with tc.tile_pool(name="head", bufs=1) as hd:
                pool_sb = pool_acc
                for grp in range(4):
                    nc.sync.dma_start(
                        out=pool_in[grp * 128:(grp + 1) * 128, :],
                        in_=pool_sb[:, grp, :])
                nc.gpsimd.collective_compute(
                    "AllReduce", mybir.AluOpType.add, replica_groups=groups,
                    ins=[pool_in.ap().opt()], outs=[pool_out.ap().opt()])
                mean = hd.tile([128, 4, 32], F32)
                for grp in range(4):
                    p2 = hd.tile([128, 32], F32, tag="p2")
                    nc.sync.dma_start(
                        out=p2[:],
                        in_=pool_out[grp * 128:(grp + 1) * 128, :])
                    nc.vector.tensor_scalar(
                        out=mean[:, grp, :], in0=p2[:],
                        scalar1=cnt_inv_t[:, grp, :], scalar2=None,
                        op0=mybir.AluOpType.mult)
                for grp in range(4):
                    trf = tp2.tile([32, 128], F32, tag="trp")
                    nc.tensor.transpose(out=trf[:], in_=mean[:, grp, :],
                                        identity=eyef_t[:])
                    trsb = hd.tile([32, 128], F32, tag="trsb")
                    nc.vector.tensor_copy(out=trsb[:], in_=trf[:])
                    ops = hp.tile([2, 128], F32, tag="hps")
                    nc.tensor.matmul(out=ops[:], lhsT=wc_t[:], rhs=trsb[:],
                                     start=True, stop=True,
                                     skip_group_check=True)
                    res = hd.tile([2, 128], F32, tag="res")
                    nc.vector.tensor_scalar(
                        out=res[:], in0=ops[:], scalar1=bc_t[:], scalar2=None,
                        op0=mybir.AluOpType.add)
                    for k in range(2):
                        nc.sync.dma_start(
                            out=out[grp * 128:(grp + 1) * 128, k:k + 1],
                            in_=res[k:k + 1, :])
    return nc


# ------------------------------------------------------------------- runner
def kernel(**inputs):
    global _LAST_RESULTS
    import os

    _install_patches()
    from concourse.bass_utils import run_bass_kernel_spmd

    per_core = _prepare(**inputs)
    nc = _build_nc()
    trace = bool(os.environ.get("GCN_TRACE"))
    kw = {}
    if trace:
        kw = dict(trace=True, trace_cores=[0, 3])
    res = run_bass_kernel_spmd(
        nc, per_core, core_ids=list(range(NCORES)), **kw)
    _LAST_RESULTS = res
    return np.asarray(res.results[0]["out"], dtype=np.float32)

